# revision 63
# baseline (speedup 1.0000x reference)
"""Self-contained Trainium2 Bass kernel for nn_Attention_37125697306831.

Multi-head attention block: B=4, H=W=48 (N=2304), C=256, 8 heads, head_dim=32,
RoPE (rotate-half), softmax attention, separate Q/K/V projections (K without
bias), output projection with bias.

Sharding: 8 cores = (batch b in 0..3) x (query half in 0..1). Each core:
  - computes Q for its 1152 queries (all heads), K/V for all 2304 keys of its
    batch, attention + output projection for its 1152 query rows.
  - no collectives; output rows are disjoint across cores.

On-chip layouts (same as the original kernel):
  - xT [ci, n], qT/kT [c, n] (head dim on partitions), V natural [n, c] with a
    33rd all-ones column per head feeding the softmax denominator.
  - scores computed transposed S.T[m keys, n queries] via row-packed K=32
    fp16 matmuls (tile_position), A@V as col-packed fp16 matmuls contracting
    over keys (K=128), normalization via per-head K=1 broadcast matmuls + DVE
    multiply, output projection consumes normalized out.T as lhsT.

Performance structure (the point of this rewrite):
  - The (scores -> exp -> A@V) chain is software-pipelined: A@V for key-tile t
    is emitted one slot after exp(t), so the PE streams scores(t+1) while the
    exp engine works on tile t. PE and the exp engines run concurrently.
  - exp is split across two engines: heads 0-3 use the ScalarE activation
    table (true exp), heads 4-7 use a Schraudolph fast-exp on the Pool engine
    (i16 = round(A*s + B) bitcast to fp16, ~3% wiggle that largely cancels in
    the softmax ratio). This halves the former ScalarE bottleneck.
  - Q/K/V projections, RoPE combines and the output projection are emitted as
    "filler" units inside the attention slot loop so they hide in PE slack.
  - Weights ship as one packed DMA; cos/sin tables are split into a critical
    head chunk + bulk remainder, issued from four different engine queues so
    the first scores start ~5us into the kernel.

All matmul operands are fp16 (PSUM accumulation fp32); elementwise math
(RoPE, exp, reciprocal, bias adds) stays fp32.
"""

import numpy as np
from collections import deque
from contextlib import ExitStack

import concourse.bass as bass
import concourse.tile as tile
from concourse import bacc, mybir
from concourse.bass_utils import run_bass_kernel_spmd

F32 = mybir.dt.float32
F16 = mybir.dt.float16
I16 = mybir.dt.int16
AF = mybir.ActivationFunctionType

B, HH, WW, C = 4, 48, 48, 256
N = HH * WW            # 2304 keys per batch
NQ = N // 2            # 1152 queries per core
NH, HD, D2 = 8, 32, 16
NT = N // 128          # 18 key m-tiles
ROPE_BASE = 10000.0
SCALE = HD ** -0.5

QCH = [(0, 512), (512, 512), (1024, 128)]                       # query chunks
KCH = [(0, 512), (512, 512), (1024, 512), (1536, 512), (2048, 256)]
ACOL = {512: 256, 128: 128}     # exp column split: ScalarE [0:a), DVE [a:cw)
NG = 4                  # 4 groups of 2 heads
VW = 33                 # V columns per head incl. the ones column
CCUT = 512              # critical-priority column cut for table DMAs

# Schraudolph fast-exp: fp16 bits of exp(x) ~= round(A*x + B). B carries an
# extra +2^10 (a uniform *2 on the fake exp) for margin against the i16<0
# cliff at very negative scores; the *2 cancels in the softmax ratio.
SCH_A = float(2 ** 10 / np.log(2.0))
SCH_B = float(2 ** 10 * (16 - 0.0579))

mul = mybir.AluOpType.mult
add_op = mybir.AluOpType.add

IN_SPECS = [
    ("xT", [C, N], F16), ("xTq", [C, NQ], F16),
    ("wpack", [128, 12 * 256], F16),          # k0|k1|kr0|kr1|q0|q1|qr0|qr1|v0|v1|o0|o1
    ("bpack", [128, 4], F32),                 # qb0|qb1|rqb0|rqb1
    ("ones", [128, 128], F16),
    ("bob", [128, C], F32),                   # Wo @ v_bias + bo, broadcast
    ("CTQ", [C, NQ], F16), ("STQ", [C, NQ], F16),
    ("CTK", [C, N], F16), ("STK", [C, N], F16),
]

W_K, W_KR, W_Q, W_QR, W_V, W_O = range(6)


def emit(tc, io, R=1, act_copy=True, schraudolph=True, act_dma=True,
         pipeline=True, split=True):
    nc = tc.nc
    ts_ = bass.ts
    ctx = ExitStack()
    with ctx:
        consts = ctx.enter_context(tc.tile_pool(name="consts", bufs=1))
        sb = ctx.enter_context(tc.tile_pool(name="sb", bufs=1))
        tmp = ctx.enter_context(tc.tile_pool(name="tmp", bufs=4))
        ptpool = ctx.enter_context(tc.tile_pool(name="pt", bufs=4))
        outpool = ctx.enter_context(tc.tile_pool(name="outT", bufs=2))
        ypool = ctx.enter_context(tc.tile_pool(name="y", bufs=3))
        rpool = ctx.enter_context(tc.tile_pool(name="recip", bufs=2))
        # PSUM (8 banks): per-head scores 2 pools x 2 x 1 bank + misc
        # (proj/V/rf/yps) 1 x 2 + av 2 x 1. All matmul outputs are
        # 2KB-bank-aligned (mid-bank PSUM matmul outputs fault the exec
        # unit on real hardware), and the two exp engines read disjoint
        # TILES (two readers of one tile serialize through dep tracking).
        scpA = ctx.enter_context(tc.tile_pool(name="scpA", bufs=2, space="PSUM"))
        scpB = ctx.enter_context(tc.tile_pool(name="scpB", bufs=2, space="PSUM"))
        mscp = ctx.enter_context(tc.tile_pool(name="mscp", bufs=1, space="PSUM"))
        avp = ctx.enter_context(tc.tile_pool(name="avp", bufs=2, space="PSUM"))

        # ---- constant tiles ----------------------------------------------
        xT0 = consts.tile([128, N], F16, tag="xT0", name="xT0")
        xT1 = consts.tile([128, N], F16, tag="xT1", name="xT1")
        xTq0 = consts.tile([128, NQ], F16, tag="xTq0", name="xTq0")
        xTq1 = consts.tile([128, NQ], F16, tag="xTq1", name="xTq1")
        wpack = consts.tile([128, 12 * 256], F16, tag="wpack", name="wpack")
        bpack = consts.tile([128, 4], F32, tag="bpack", name="bpack")
        ones = consts.tile([128, 128], F16, tag="ones", name="ones")
        bob = consts.tile([128, C], F32, tag="bob", name="bob")
        ctq = [consts.tile([128, NQ], F16, tag=f"ctq{i}", name=f"ctq{i}") for i in range(2)]
        stq = [consts.tile([128, NQ], F16, tag=f"stq{i}", name=f"stq{i}") for i in range(2)]
        ctk = [consts.tile([128, N], F16, tag=f"ctk{i}", name=f"ctk{i}") for i in range(2)]
        stk = [consts.tile([128, N], F16, tag=f"stk{i}", name=f"stk{i}") for i in range(2)]

        def WS(j, half, cg):        # [128, 128] stationary slice
            o = 512 * j + 256 * half + 128 * cg
            return wpack[:, o:o + 128]

        def WF(j, half):            # [128, 256] moving slice (wv / wo)
            o = 512 * j + 256 * half
            return wpack[:, o:o + 256]

        qb = [bpack[:, 0:1], bpack[:, 1:2]]
        rqb = [bpack[:, 2:3], bpack[:, 3:4]]

        # ---- loads: critical first, spread across engine queues ----------
        # SP: K-projection weights first, then x columns, then Q-side.
        nc.sync.dma_start(wpack[:, 0:1024], io["wpack"][:, 0:1024])      # wk|wkr
        nc.sync.dma_start(xT0[:, 0:CCUT], io["xT"][0:128, 0:CCUT])
        nc.sync.dma_start(xT1[:, 0:CCUT], io["xT"][128:256, 0:CCUT])
        nc.sync.dma_start(wpack[:, 1024:], io["wpack"][:, 1024:])        # wq|wqr|wv|wo
        nc.sync.dma_start(xTq0[:], io["xTq"][0:128, :])
        nc.sync.dma_start(xTq1[:], io["xTq"][128:256, :])
        nc.sync.dma_start(bpack[:], io["bpack"][:])
        # Rope-table head chunks (needed by the first combines).
        dma_eng = nc.scalar if act_dma else nc.sync
        dma_eng.dma_start(ctk[0][:, 0:CCUT], io["CTK"][0:128, 0:CCUT])
        dma_eng.dma_start(ctk[1][:, 0:CCUT], io["CTK"][128:256, 0:CCUT])
        dma_eng.dma_start(stk[0][:, 0:CCUT], io["STK"][0:128, 0:CCUT])
        dma_eng.dma_start(stk[1][:, 0:CCUT], io["STK"][128:256, 0:CCUT])
        dma_eng.dma_start(ctq[0][:, 0:CCUT], io["CTQ"][0:128, 0:CCUT])
        dma_eng.dma_start(ctq[1][:, 0:CCUT], io["CTQ"][128:256, 0:CCUT])
        dma_eng.dma_start(stq[0][:, 0:CCUT], io["STQ"][0:128, 0:CCUT])
        dma_eng.dma_start(stq[1][:, 0:CCUT], io["STQ"][128:256, 0:CCUT])
        # SP bulk, ordered by first use: the columns feeding the early
        # K-projection fillers (512:1024) land before the long tail.
        C2 = 1024
        nc.sync.dma_start(xT0[:, CCUT:C2], io["xT"][0:128, CCUT:C2])
        nc.sync.dma_start(xT1[:, CCUT:C2], io["xT"][128:256, CCUT:C2])
        nc.sync.dma_start(ctk[0][:, CCUT:C2], io["CTK"][0:128, CCUT:C2])
        nc.sync.dma_start(ctk[1][:, CCUT:C2], io["CTK"][128:256, CCUT:C2])
        nc.sync.dma_start(stk[0][:, CCUT:C2], io["STK"][0:128, CCUT:C2])
        nc.sync.dma_start(stk[1][:, CCUT:C2], io["STK"][128:256, CCUT:C2])
        nc.sync.dma_start(xT0[:, C2:], io["xT"][0:128, C2:])
        nc.sync.dma_start(xT1[:, C2:], io["xT"][128:256, C2:])
        nc.sync.dma_start(ctk[0][:, C2:], io["CTK"][0:128, C2:])
        nc.sync.dma_start(ctk[1][:, C2:], io["CTK"][128:256, C2:])
        nc.sync.dma_start(stk[0][:, C2:], io["STK"][0:128, C2:])
        nc.sync.dma_start(stk[1][:, C2:], io["STK"][128:256, C2:])
        nc.sync.dma_start(ctq[0][:, CCUT:], io["CTQ"][0:128, CCUT:])
        nc.sync.dma_start(ctq[1][:, CCUT:], io["CTQ"][128:256, CCUT:])
        nc.sync.dma_start(stq[0][:, CCUT:], io["STQ"][0:128, CCUT:])
        nc.sync.dma_start(stq[1][:, CCUT:], io["STQ"][128:256, CCUT:])
        nc.sync.dma_start(ones[:], io["ones"][:])
        nc.sync.dma_start(bob[:], io["bob"][:])

        if R > 1:
            loop_ctx = tc.For_i(0, R, 1)
            loop_ctx.__enter__()

        # ---- persistent SBUF tiles ---------------------------------------
        qT = [sb.tile([128, NQ], F16, tag=f"qT{i}", name=f"qT{i}") for i in range(2)]
        kT = [sb.tile([128, N], F16, tag=f"kT{i}", name=f"kT{i}") for i in range(2)]
        vsb = sb.tile([128, NT * NH * VW], F16, tag="v", name="vsb")
        nc.gpsimd.memset(vsb[:, 0:4 * NH * VW], 1.0)
        nc.gpsimd.memset(vsb[:, 4 * NH * VW:], 1.0)

        # ---- unit emitters -----------------------------------------------
        def proj_qk(dst, wj, wrj, bias, rbias, xa, xb, ct, st, cg, off, cw):
            # dst[cg][:, off:off+cw] = (w.T x + b) * ct + (wr.T x + rb) * st
            ps = mscp.tile([128, 2, 512], F32, tag="ms", name="ps")
            nc.tensor.matmul(ps[:, 0, :cw], WS(wj, 0, cg), xa[:, off:off + cw],
                             start=True, stop=False)
            nc.tensor.matmul(ps[:, 0, :cw], WS(wj, 1, cg), xb[:, off:off + cw],
                             start=False, stop=True)
            nc.tensor.matmul(ps[:, 1, :cw], WS(wrj, 0, cg), xa[:, off:off + cw],
                             start=True, stop=False)
            nc.tensor.matmul(ps[:, 1, :cw], WS(wrj, 1, cg), xb[:, off:off + cw],
                             start=False, stop=True)
            b0 = bias[cg] if bias is not None else 0.0
            b1 = rbias[cg] if rbias is not None else 0.0
            t1 = tmp.tile([128, 512], F32, tag="t1", name="t1")
            nc.vector.scalar_tensor_tensor(
                t1[:, 0:cw], ps[:, 0, 0:cw], b0,
                ct[cg][:, off:off + cw], op0=add_op, op1=mul)
            t2 = tmp.tile([128, 512], F32, tag="t2", name="t2")
            nc.vector.scalar_tensor_tensor(
                t2[:, 0:cw], ps[:, 1, 0:cw], b1,
                st[cg][:, off:off + cw], op0=add_op, op1=mul)
            nc.vector.tensor_add(dst[cg][:, off:off + cw],
                                 t1[:, 0:cw], t2[:, 0:cw])

        def k_unit(cg, j):
            proj_qk(kT, W_K, W_KR, None, None, xT0, xT1, ctk, stk, cg, *KCH[j])

        def q_unit(cg, j):
            proj_qk(qT, W_Q, W_QR, qb, rqb, xTq0, xTq1, ctq, stq, cg, *QCH[j])

        def v_unit(t):
            # V natural [keys, c] in 33-wide head blocks; the 33rd column
            # stays 1.0 from the memset. v_bias is folded into bob host-side.
            ps = mscp.tile([128, 2, 512], F32, tag="ms", name="vps")
            nc.tensor.matmul(ps[:, 0, :C], xT0[:, ts_(t, 128)], WF(W_V, 0),
                             start=True, stop=False)
            nc.tensor.matmul(ps[:, 0, :C], xT1[:, ts_(t, 128)], WF(W_V, 1),
                             start=False, stop=True)
            vdst = vsb[:, t * NH * VW:(t + 1) * NH * VW]
            vdst = vdst.rearrange("p (h c) -> p h c", c=VW)
            psrc = ps[:, 0, 0:C].rearrange("p (h c) -> p h c", c=HD)
            if act_copy:
                nc.scalar.copy(vdst[:, :, 0:HD], psrc[:])
            else:
                nc.vector.tensor_copy(vdst[:, :, 0:HD], psrc[:])

        av_tiles = {}
        rs_tiles = {}
        oT_tiles = {}

        def fin_a(ci, g):
            # reciprocal of the fused softmax-denominator rows (32, 96)
            qoff, cw = QCH[ci]
            av = av_tiles[(ci, g)]
            rsb = rpool.tile([128, 512], F16, tag="rs", name="rsb")
            with nc.allow_low_precision("fp16 softmax scale rows"):
                for gi in range(2):
                    r = 64 * gi + 32
                    nc.vector.reciprocal(rsb[r:r + 1, 0:cw], av[r:r + 1, 0:cw])
            rs_tiles[(ci, g)] = rsb

        def fin_b(ci, g):
            # broadcast recip via K=1 matmuls, then normalize into oT
            qoff, cw = QCH[ci]
            av = av_tiles.pop((ci, g))
            rsb = rs_tiles.pop((ci, g))
            rf = mscp.tile([128, 2, 512], F32, tag="ms", name="rf")
            for gi in range(2):
                r = 64 * gi + 32
                nc.tensor.matmul(rf[ts_(gi, 64), 0, 0:cw][0:32, :],
                                 ones[r:r + 1, 0:32],
                                 rsb[r:r + 1, 0:cw],
                                 start=True, stop=True,
                                 tile_position=(r, 64 * gi),
                                 skip_group_check=True)
            rfsb = rpool.tile([128, 512], F32, tag="rfsb", name="rfsb")
            for gi in range(2):
                if act_copy:
                    nc.scalar.copy(rfsb[64 * gi:64 * gi + 32, 0:cw],
                                   rf[64 * gi:64 * gi + 32, 0, 0:cw])
                else:
                    nc.vector.tensor_copy(rfsb[64 * gi:64 * gi + 32, 0:cw],
                                          rf[64 * gi:64 * gi + 32, 0, 0:cw])
            oT0, oT1 = oT_tiles[ci]
            dst = oT0 if g < 2 else oT1
            d0 = 64 * (g % 2)
            for gi in range(2):
                nc.vector.tensor_mul(dst[d0 + 32 * gi:d0 + 32 * gi + 32, 0:cw],
                                     av[64 * gi:64 * gi + 32, 0:cw],
                                     rfsb[64 * gi:64 * gi + 32, 0:cw])

        def outproj(ci, s):
            qoff, cw = QCH[ci]
            oT0, oT1 = oT_tiles[ci]
            yps = mscp.tile([128, 2, 512], F32, tag="ms", name="yps")
            nc.tensor.matmul(yps[:, 0, :C], oT0[:, ts_(s, 128)], WF(W_O, 0),
                             start=True, stop=False)
            nc.tensor.matmul(yps[:, 0, :C], oT1[:, ts_(s, 128)], WF(W_O, 1),
                             start=False, stop=True)
            ysb = ypool.tile([128, C], F32, tag="y", name="ysb")
            nc.vector.tensor_add(ysb[:], yps[:, 0, 0:C], bob[:])
            nc.sync.dma_start(io["y"][qoff + 128 * s: qoff + 128 * (s + 1), :],
                              ysb[:])

        delayed = []

        def emit_av(ci, g, t, ptA, ptB):
            qoff, cw = QCH[ci]
            if t == 0:
                av_tiles[(ci, g)] = avp.tile([128, 512], F32, tag="av", name="av")
            av = av_tiles[(ci, g)]
            for gi in range(2):
                h = 2 * g + gi
                vslice = vsb[:, (t * NH + h) * VW:(t * NH + h + 1) * VW]
                if gi == 1 and ptB is not None:
                    pts = (ptB[:, 0:cw].bitcast(F16)
                           if ptB.dtype == I16 else ptB[:, 0:cw])
                else:
                    pts = ptA[:, gi, 0:cw]
                nc.tensor.matmul(
                    av[64 * gi:64 * gi + VW, 0:cw], vslice, pts,
                    start=(t == 0), stop=(t == NT - 1),
                    skip_group_check=True)
            if t == NT - 1:
                fin_a(ci, g)
                delayed.append([1, (fin_b, ci, g)])

        def run_delayed():
            for item in list(delayed):
                item[0] -= 1
                if item[0] <= 0:
                    fn, *args = item[1]
                    fn(*args)
                    delayed.remove(item)

        # ---- filler schedule --------------------------------------------
        fillers = {}

        def addf(ci, g, t, fn, *args):
            fillers.setdefault((ci, g, t), []).append((fn, args))

        for j in range(4):                       # K cg0 chunks 1..4
            addf(0, 0, j, k_unit, 0, j + 1)
        for j in range(4, NT):                   # V tiles 4..17
            addf(0, 0, j - 1, v_unit, j)
        for i in range(5):                       # K cg1 chunks 0..4
            addf(0, 1, 2 * i, k_unit, 1, i)
        addf(0, 1, 10, q_unit, 1, 0)             # Q cg1 chunk0
        addf(0, 2, 0, q_unit, 0, 1)
        addf(0, 2, 2, q_unit, 1, 1)
        addf(1, 2, 0, q_unit, 0, 2)
        addf(1, 2, 2, q_unit, 1, 2)
        for s in range(4):                       # out-proj of previous chunk
            addf(1, 0, 2 + 2 * s, outproj, 0, s)
            addf(2, 0, 2 + 2 * s, outproj, 1, s)

        # ---- pre-attention head: minimal work to start scores ------------
        k_unit(0, 0)
        q_unit(0, 0)
        for t in range(4):
            v_unit(t)

        # ---- pipelined attention (A@V runs two slots behind scores) ------
        pending = deque()
        for ci in range(3):
            qoff, cw = QCH[ci]
            for g in range(NG):
                for t in range(NT):
                    if g == 0 and t == 0:
                        oT_tiles[ci] = (
                            outpool.tile([128, 512], F16, tag="o0", name="oT0"),
                            outpool.tile([128, 512], F16, tag="o1", name="oT1"))
                    # exp split by head: gi=0 on ScalarE (true exp), gi=1 on
                    # DVE (Schraudolph; GPSIMD cannot read PSUM). Each head's
                    # softmax rows see a single exp flavor, so the
                    # Schraudolph 2x scale cancels in normalization. Each gi
                    # gets its own single-bank PSUM tile and its own output
                    # tile — sharing either serializes the engines through
                    # dependency tracking. The small last chunk runs entirely
                    # on ScalarE to unload DVE.
                    dve_gi = bool(split)
                    sc0 = scpA.tile([128, 512], F32, tag="sc0", name="sc0")
                    sc1 = scpB.tile([128, 512], F32, tag="sc1", name="sc1")
                    ptA = ptpool.tile([128, 2, 512], F16, tag="ptA",
                                      name="ptA")
                    ptB = (ptpool.tile([128, 512],
                                       I16 if schraudolph else F16,
                                       tag="ptB", name="ptB")
                           if dve_gi else None)
                    for gi in range(2):
                        h = 2 * g + gi
                        cg, hh = h // 4, h % 4
                        nc.tensor.matmul(
                            (sc0 if gi == 0 else sc1)[:, 0:cw],
                            kT[cg][ts_(hh, 32), ts_(t, 128)],
                            qT[cg][ts_(hh, 32), qoff:qoff + cw],
                            start=True, stop=True, tile_position=(32 * hh, 0))
                    nc.scalar.activation(ptA[:, 0, 0:cw],
                                         sc0[:, 0:cw], AF.Exp)
                    if dve_gi:
                        if schraudolph:
                            nc.vector.tensor_scalar(
                                ptB[:, 0:cw], sc1[:, 0:cw],
                                SCH_A, SCH_B, mul, add_op)
                        else:
                            nc.scalar.activation(ptB[:, 0:cw],
                                                 sc1[:, 0:cw], AF.Exp)
                    else:
                        nc.scalar.activation(ptA[:, 1, 0:cw],
                                             sc1[:, 0:cw], AF.Exp)
                    run_delayed()
                    depth = 2 if pipeline else 0
                    while len(pending) >= max(depth, 1):
                        emit_av(*pending.popleft())
                        if not pipeline:
                            run_delayed()
                    pending.append((ci, g, t, ptA, ptB))
                    if not pipeline:
                        emit_av(*pending.popleft())
                        run_delayed()
                    for fn, args in fillers.get((ci, g, t), []):
                        fn(*args)
        while pending:
            emit_av(*pending.popleft())
            run_delayed()
        run_delayed()          # fin_b of (2, 3)
        outproj(2, 0)

        if R > 1:
            loop_ctx.__exit__(None, None, None)


def build_nc(R=1, act_copy=True, schraudolph=True, act_dma=True,
             pipeline=True, split=True):
    nc = bacc.Bacc("TRN2", target_bir_lowering=False, debug=False,
                   enable_asserts=True, num_devices=8)
    io = {}
    for name, shape, dt in IN_SPECS:
        io[name] = nc.dram_tensor(name, shape, dt, kind="ExternalInput").ap()
    io["y"] = nc.dram_tensor("y", [NQ, C], F32, kind="ExternalOutput").ap()

    with tile.TileContext(nc) as tc:
        emit(tc, io, R=R, act_copy=act_copy, schraudolph=schraudolph,
             act_dma=act_dma, pipeline=pipeline, split=split)
    nc.compile()
    return nc


def host_inputs(x, Wq, q_bias, Wk, Wv, v_bias, Wo, bo):
    """Build the per-core input maps (host-side sharding + layout prep)."""
    xf = np.ascontiguousarray(x.reshape(B, N, C))

    inv_freq = 1.0 / (ROPE_BASE ** (np.arange(0, HD, 2, dtype=np.float64) / HD))
    pos = np.arange(N, dtype=np.float64)
    ang = pos[:, None] * inv_freq[None, :]          # [N, 16]
    cos_t, sin_t = np.cos(ang), np.sin(ang)         # [N, 16]
    # channel c -> within-head index jj = c % 32, freq f = jj % 16
    jj = np.arange(C) % HD
    f = jj % D2
    CT = cos_t[:, f].T                              # [C, N] float64
    ST = sin_t[:, f].T

    # signed rotate-half permutation RM [C, C]: partner = RM @ q
    RM = np.zeros((C, C), dtype=np.float64)
    for p in range(C):
        j = p % HD
        if j < D2:
            RM[p, p + D2] = -1.0                    # partner[p] = -q[p+16]
        else:
            RM[p, p - D2] = 1.0                     # partner[p] = +q[p-16]

    Wq64, Wk64 = Wq.astype(np.float64), Wk.astype(np.float64)
    Wqr = RM @ Wq64                                 # rotated projections
    Wkr = RM @ Wk64
    rqb = RM @ q_bias.astype(np.float64)

    f16 = lambda a: np.ascontiguousarray(a, dtype=np.float16)
    f32 = lambda a: np.ascontiguousarray(a, dtype=np.float32)

    halves = []
    for M in (Wk64.T, Wkr.T, Wq64.T, Wqr.T, Wv.T, Wo.T):
        halves.append(np.asarray(M[0:128, :], np.float64))
        halves.append(np.asarray(M[128:256, :], np.float64))
    wpack = f16(np.concatenate(halves, axis=1))     # [128, 12*256]
    bpack = f32(np.stack([q_bias[0:128], q_bias[128:256],
                          rqb[0:128], rqb[128:256]], axis=1))
    # v_bias folded through the output projection: attn weights sum to 1
    # after normalization, so out = attn@(V+vb) = attn@V + vb exactly.
    bob = Wo.astype(np.float64) @ v_bias.astype(np.float64) + bo

    common = {
        "wpack": wpack, "bpack": bpack,
        "ones": np.ones((128, 128), dtype=np.float16),
        "bob": f32(np.broadcast_to(bob, (128, C))),
        "CTK": f16(CT), "STK": f16(ST),
    }
    in_maps = []
    for core in range(8):
        b, qhalf = core // 2, core % 2
        qoff = qhalf * NQ
        xT = xf[b].T
        m = dict(common)
        m["xT"] = f16(xT)
        m["xTq"] = f16(xT[:, qoff:qoff + NQ])
        m["CTQ"] = f16(CT[:, qoff:qoff + NQ] * SCALE)
        m["STQ"] = f16(ST[:, qoff:qoff + NQ] * SCALE)
        in_maps.append(m)
    return in_maps


_NC_CACHE = {}


def get_nc(R=1):
    if R not in _NC_CACHE:
        _NC_CACHE[R] = build_nc(R)
    return _NC_CACHE[R]


def kernel(**inputs):
    inputs = {k: np.asarray(v, dtype=np.float32) for k, v in inputs.items()}
    in_maps = host_inputs(**inputs)
    nc = get_nc()
    res = run_bass_kernel_spmd(nc, in_maps, core_ids=list(range(8)))
    out = np.empty((B, N, C), dtype=np.float32)
    for core in range(8):
        b, qhalf = core // 2, core % 2
        qoff = qhalf * NQ
        out[b, qoff:qoff + NQ, :] = res.results[core]["y"]
    return out.reshape(B, HH, WW, C)


# revision 69
# speedup vs baseline: 1.0968x; 1.0968x over previous
"""Self-contained Trainium2 Bass kernel for nn_Attention_37125697306831.

Multi-head attention block: B=4, H=W=48 (N=2304), C=256, 8 heads, head_dim=32,
RoPE (rotate-half), softmax attention, separate Q/K/V projections (K without
bias), output projection with bias.

Sharding: 8 cores = (batch b in 0..3) x (query half in 0..1). Each core:
  - computes Q for its 1152 queries (all heads), K/V for all 2304 keys of its
    batch, attention + output projection for its 1152 query rows.
  - no collectives; output rows are disjoint across cores.

On-chip layouts (same as the original kernel):
  - xT [ci, n], qT/kT [c, n] (head dim on partitions), V natural [n, c] with a
    33rd all-ones column per head feeding the softmax denominator.
  - scores computed transposed S.T[m keys, n queries] via row-packed K=32
    fp16 matmuls (tile_position), A@V as col-packed fp16 matmuls contracting
    over keys (K=128), normalization via per-head K=1 broadcast matmuls + DVE
    multiply, output projection consumes normalized out.T as lhsT.

Performance structure (the point of this rewrite):
  - The (scores -> exp -> A@V) chain is software-pipelined: A@V for key-tile t
    is emitted two slots after exp(t), so the PE streams scores(t+1..t+2)
    while ScalarE works on tile t, and the two engines run concurrently.
  - Q/K/V projections, RoPE combines and the output projection are emitted as
    "filler" units inside the attention slot loop so they hide in PE/DVE
    slack; softmax finalization (reciprocal / broadcast / normalize) is
    emitted with a one-slot delay for the same reason.
  - v_bias is folded into the output-projection bias host-side (attention
    weights sum to 1 after normalization, so attn@(V+vb) = attn@V + vb).
  - Weights ship as one packed DMA; cos/sin tables are split into a critical
    head chunk + bulk remainder, issued from two engine queues so the first
    scores start a few us into the kernel.

Hardware findings baked in (flags preserve the faster-but-invalid variants):
  - GPSIMD/Pool cannot access PSUM (BIR verifier rejects it).
  - Matmul PSUM outputs must be 2KB-bank-aligned; mid-bank outputs fault the
    exec unit (NRT_EXEC_UNIT_UNRECOVERABLE).
  - int16 engine-output conversion runs ~6-10x slower than the cost model on
    every engine, which kills the Schraudolph fast-exp split (schraudolph
    flag); exp therefore runs entirely on ScalarE, one 2-head 1024-element
    activation per slot, which is the pace-setting engine.
  - Two readers of one PSUM tile, or two writers of one SBUF tile (even for
    disjoint subtiles via bitcast), serialize through dependency tracking.

All matmul operands are fp16 (PSUM accumulation fp32); elementwise math
(RoPE, exp, reciprocal, bias adds) stays fp32.
"""

import numpy as np
from collections import deque
from contextlib import ExitStack

import concourse.bass as bass
import concourse.tile as tile
from concourse import bacc, mybir
from concourse.bass_utils import run_bass_kernel_spmd

F32 = mybir.dt.float32
F16 = mybir.dt.float16
I16 = mybir.dt.int16
AF = mybir.ActivationFunctionType

B, HH, WW, C = 4, 48, 48, 256
N = HH * WW            # 2304 keys per batch
NQ = N // 2            # 1152 queries per core
NH, HD, D2 = 8, 32, 16
NT = N // 128          # 18 key m-tiles
ROPE_BASE = 10000.0
SCALE = HD ** -0.5

QCH = [(0, 512), (512, 512), (1024, 128)]                       # query chunks
KCH = [(0, 512), (512, 512), (1024, 512), (1536, 512), (2048, 256)]
ACOL = {512: 256, 128: 128}     # exp column split: ScalarE [0:a), DVE [a:cw)
NG = 4                  # 4 groups of 2 heads
VW = 33                 # V columns per head incl. the ones column
CCUT = 512              # critical-priority column cut for table DMAs

# Schraudolph fast-exp: fp16 bits of exp(x) ~= round(A*x + B). B carries an
# extra +2^10 (a uniform *2 on the fake exp) for margin against the i16<0
# cliff at very negative scores; the *2 cancels in the softmax ratio.
SCH_A = float(2 ** 10 / np.log(2.0))
SCH_B = float(2 ** 10 * (16 - 0.0579))

mul = mybir.AluOpType.mult
add_op = mybir.AluOpType.add

IN_SPECS = [
    ("xT", [C, N], F16), ("xTq", [C, NQ], F16),
    ("wpack", [128, 12 * 256], F16),          # k0|k1|kr0|kr1|q0|q1|qr0|qr1|v0|v1|o0|o1
    ("bpack", [128, 4], F32),                 # qb0|qb1|rqb0|rqb1
    ("ones", [128, 128], F16),
    ("bob", [128, C], F32),                   # Wo @ v_bias + bo, broadcast
    ("CTQ", [C, NQ], F16), ("STQ", [C, NQ], F16),
    ("CTK", [C, N], F16), ("STK", [C, N], F16),
]

W_K, W_KR, W_Q, W_QR, W_V, W_O = range(6)


def emit(tc, io, R=1, act_copy=False, schraudolph=False, act_dma=True,
         pipeline=True, split=False):
    nc = tc.nc
    ts_ = bass.ts
    ctx = ExitStack()
    with ctx:
        consts = ctx.enter_context(tc.tile_pool(name="consts", bufs=1))
        sb = ctx.enter_context(tc.tile_pool(name="sb", bufs=1))
        tmp = ctx.enter_context(tc.tile_pool(name="tmp", bufs=4))
        ptpool = ctx.enter_context(tc.tile_pool(name="pt", bufs=4))
        outpool = ctx.enter_context(tc.tile_pool(name="outT", bufs=2))
        ypool = ctx.enter_context(tc.tile_pool(name="y", bufs=3))
        rpool = ctx.enter_context(tc.tile_pool(name="recip", bufs=2))
        # PSUM (8 banks): scores 2 x 2 banks + misc (proj/V/rf/yps) 1 x 2 +
        # av 2 x 1. All matmul outputs are 2KB-bank-aligned (mid-bank PSUM
        # matmul outputs fault the exec unit on real hardware). exp runs as
        # ONE 1024-element ScalarE instruction per slot: the ~475ns real
        # per-instruction ScalarE overhead makes instruction count the
        # dominant cost, so never split the exp.
        scpA = ctx.enter_context(tc.tile_pool(name="scpA", bufs=2, space="PSUM"))
        mscp = ctx.enter_context(tc.tile_pool(name="mscp", bufs=1, space="PSUM"))
        avp = ctx.enter_context(tc.tile_pool(name="avp", bufs=2, space="PSUM"))

        # ---- constant tiles ----------------------------------------------
        xT0 = consts.tile([128, N], F16, tag="xT0", name="xT0")
        xT1 = consts.tile([128, N], F16, tag="xT1", name="xT1")
        xTq0 = consts.tile([128, NQ], F16, tag="xTq0", name="xTq0")
        xTq1 = consts.tile([128, NQ], F16, tag="xTq1", name="xTq1")
        wpack = consts.tile([128, 12 * 256], F16, tag="wpack", name="wpack")
        bpack = consts.tile([128, 4], F32, tag="bpack", name="bpack")
        ones = consts.tile([128, 128], F16, tag="ones", name="ones")
        bob = consts.tile([128, C], F32, tag="bob", name="bob")
        ctq = [consts.tile([128, NQ], F16, tag=f"ctq{i}", name=f"ctq{i}") for i in range(2)]
        stq = [consts.tile([128, NQ], F16, tag=f"stq{i}", name=f"stq{i}") for i in range(2)]
        ctk = [consts.tile([128, N], F16, tag=f"ctk{i}", name=f"ctk{i}") for i in range(2)]
        stk = [consts.tile([128, N], F16, tag=f"stk{i}", name=f"stk{i}") for i in range(2)]

        def WS(j, half, cg):        # [128, 128] stationary slice
            o = 512 * j + 256 * half + 128 * cg
            return wpack[:, o:o + 128]

        def WF(j, half):            # [128, 256] moving slice (wv / wo)
            o = 512 * j + 256 * half
            return wpack[:, o:o + 256]

        qb = [bpack[:, 0:1], bpack[:, 1:2]]
        rqb = [bpack[:, 2:3], bpack[:, 3:4]]

        # ---- loads: critical first, spread across engine queues ----------
        # SP: K-projection weights first, then x columns, then Q-side.
        nc.sync.dma_start(wpack[:, 0:1024], io["wpack"][:, 0:1024])      # wk|wkr
        nc.sync.dma_start(xT0[:, 0:CCUT], io["xT"][0:128, 0:CCUT])
        nc.sync.dma_start(xT1[:, 0:CCUT], io["xT"][128:256, 0:CCUT])
        nc.sync.dma_start(wpack[:, 1024:], io["wpack"][:, 1024:])        # wq|wqr|wv|wo
        nc.sync.dma_start(xTq0[:], io["xTq"][0:128, :])
        nc.sync.dma_start(xTq1[:], io["xTq"][128:256, :])
        nc.sync.dma_start(bpack[:], io["bpack"][:])
        # Rope-table head chunks (needed by the first combines).
        dma_eng = nc.scalar if act_dma else nc.sync
        dma_eng.dma_start(ctk[0][:, 0:CCUT], io["CTK"][0:128, 0:CCUT])
        dma_eng.dma_start(ctk[1][:, 0:CCUT], io["CTK"][128:256, 0:CCUT])
        dma_eng.dma_start(stk[0][:, 0:CCUT], io["STK"][0:128, 0:CCUT])
        dma_eng.dma_start(stk[1][:, 0:CCUT], io["STK"][128:256, 0:CCUT])
        dma_eng.dma_start(ctq[0][:, 0:CCUT], io["CTQ"][0:128, 0:CCUT])
        dma_eng.dma_start(ctq[1][:, 0:CCUT], io["CTQ"][128:256, 0:CCUT])
        dma_eng.dma_start(stq[0][:, 0:CCUT], io["STQ"][0:128, 0:CCUT])
        dma_eng.dma_start(stq[1][:, 0:CCUT], io["STQ"][128:256, 0:CCUT])
        # SP bulk, ordered by first use: the columns feeding the early
        # K-projection fillers (512:1024) land before the long tail.
        C2 = 1024
        nc.sync.dma_start(xT0[:, CCUT:C2], io["xT"][0:128, CCUT:C2])
        nc.sync.dma_start(xT1[:, CCUT:C2], io["xT"][128:256, CCUT:C2])
        nc.sync.dma_start(ctk[0][:, CCUT:C2], io["CTK"][0:128, CCUT:C2])
        nc.sync.dma_start(ctk[1][:, CCUT:C2], io["CTK"][128:256, CCUT:C2])
        nc.sync.dma_start(stk[0][:, CCUT:C2], io["STK"][0:128, CCUT:C2])
        nc.sync.dma_start(stk[1][:, CCUT:C2], io["STK"][128:256, CCUT:C2])
        nc.sync.dma_start(xT0[:, C2:], io["xT"][0:128, C2:])
        nc.sync.dma_start(xT1[:, C2:], io["xT"][128:256, C2:])
        nc.sync.dma_start(ctk[0][:, C2:], io["CTK"][0:128, C2:])
        nc.sync.dma_start(ctk[1][:, C2:], io["CTK"][128:256, C2:])
        nc.sync.dma_start(stk[0][:, C2:], io["STK"][0:128, C2:])
        nc.sync.dma_start(stk[1][:, C2:], io["STK"][128:256, C2:])
        nc.sync.dma_start(ctq[0][:, CCUT:], io["CTQ"][0:128, CCUT:])
        nc.sync.dma_start(ctq[1][:, CCUT:], io["CTQ"][128:256, CCUT:])
        nc.sync.dma_start(stq[0][:, CCUT:], io["STQ"][0:128, CCUT:])
        nc.sync.dma_start(stq[1][:, CCUT:], io["STQ"][128:256, CCUT:])
        nc.sync.dma_start(ones[:], io["ones"][:])
        nc.sync.dma_start(bob[:], io["bob"][:])

        if R > 1:
            loop_ctx = tc.For_i(0, R, 1)
            loop_ctx.__enter__()

        # ---- persistent SBUF tiles ---------------------------------------
        qT = [sb.tile([128, NQ], F16, tag=f"qT{i}", name=f"qT{i}") for i in range(2)]
        kT = [sb.tile([128, N], F16, tag=f"kT{i}", name=f"kT{i}") for i in range(2)]
        vsb = sb.tile([128, NT * NH * VW], F16, tag="v", name="vsb")
        nc.gpsimd.memset(vsb[:, 0:4 * NH * VW], 1.0)
        nc.gpsimd.memset(vsb[:, 4 * NH * VW:], 1.0)

        # ---- unit emitters -----------------------------------------------
        def proj_qk(dst, wj, wrj, bias, rbias, xa, xb, ct, st, cg, off, cw):
            # dst[cg][:, off:off+cw] = (w.T x + b) * ct + (wr.T x + rb) * st
            ps = mscp.tile([128, 2, 512], F32, tag="ms", name="ps")
            nc.tensor.matmul(ps[:, 0, :cw], WS(wj, 0, cg), xa[:, off:off + cw],
                             start=True, stop=False)
            nc.tensor.matmul(ps[:, 0, :cw], WS(wj, 1, cg), xb[:, off:off + cw],
                             start=False, stop=True)
            nc.tensor.matmul(ps[:, 1, :cw], WS(wrj, 0, cg), xa[:, off:off + cw],
                             start=True, stop=False)
            nc.tensor.matmul(ps[:, 1, :cw], WS(wrj, 1, cg), xb[:, off:off + cw],
                             start=False, stop=True)
            b0 = bias[cg] if bias is not None else 0.0
            b1 = rbias[cg] if rbias is not None else 0.0
            t1 = tmp.tile([128, 512], F32, tag="t1", name="t1")
            nc.vector.scalar_tensor_tensor(
                t1[:, 0:cw], ps[:, 0, 0:cw], b0,
                ct[cg][:, off:off + cw], op0=add_op, op1=mul)
            t2 = tmp.tile([128, 512], F32, tag="t2", name="t2")
            nc.vector.scalar_tensor_tensor(
                t2[:, 0:cw], ps[:, 1, 0:cw], b1,
                st[cg][:, off:off + cw], op0=add_op, op1=mul)
            nc.vector.tensor_add(dst[cg][:, off:off + cw],
                                 t1[:, 0:cw], t2[:, 0:cw])

        def k_unit(cg, j):
            proj_qk(kT, W_K, W_KR, None, None, xT0, xT1, ctk, stk, cg, *KCH[j])

        def q_unit(cg, j):
            proj_qk(qT, W_Q, W_QR, qb, rqb, xTq0, xTq1, ctq, stq, cg, *QCH[j])

        def v_unit(t):
            # V natural [keys, c] in 33-wide head blocks; the 33rd column
            # stays 1.0 from the memset. v_bias is folded into bob host-side.
            ps = mscp.tile([128, 2, 512], F32, tag="ms", name="vps")
            nc.tensor.matmul(ps[:, 0, :C], xT0[:, ts_(t, 128)], WF(W_V, 0),
                             start=True, stop=False)
            nc.tensor.matmul(ps[:, 0, :C], xT1[:, ts_(t, 128)], WF(W_V, 1),
                             start=False, stop=True)
            vdst = vsb[:, t * NH * VW:(t + 1) * NH * VW]
            vdst = vdst.rearrange("p (h c) -> p h c", c=VW)
            psrc = ps[:, 0, 0:C].rearrange("p (h c) -> p h c", c=HD)
            if act_copy:
                nc.scalar.copy(vdst[:, :, 0:HD], psrc[:])
            else:
                nc.vector.tensor_copy(vdst[:, :, 0:HD], psrc[:])

        av_tiles = {}
        rs_tiles = {}
        oT_tiles = {}

        def fin_a(ci, g):
            # reciprocal of the fused softmax-denominator rows (32, 96)
            qoff, cw = QCH[ci]
            av = av_tiles[(ci, g)]
            rsb = rpool.tile([128, 512], F16, tag="rs", name="rsb")
            with nc.allow_low_precision("fp16 softmax scale rows"):
                for gi in range(2):
                    r = 64 * gi + 32
                    nc.vector.reciprocal(rsb[r:r + 1, 0:cw], av[r:r + 1, 0:cw])
            rs_tiles[(ci, g)] = rsb

        def fin_b(ci, g):
            # broadcast recip via K=1 matmuls, then normalize into oT
            qoff, cw = QCH[ci]
            av = av_tiles.pop((ci, g))
            rsb = rs_tiles.pop((ci, g))
            rf = mscp.tile([128, 2, 512], F32, tag="ms", name="rf")
            for gi in range(2):
                r = 64 * gi + 32
                nc.tensor.matmul(rf[ts_(gi, 64), 0, 0:cw][0:32, :],
                                 ones[r:r + 1, 0:32],
                                 rsb[r:r + 1, 0:cw],
                                 start=True, stop=True,
                                 tile_position=(r, 64 * gi),
                                 skip_group_check=True)
            rfsb = rpool.tile([128, 512], F32, tag="rfsb", name="rfsb")
            for gi in range(2):
                if act_copy:
                    nc.scalar.copy(rfsb[64 * gi:64 * gi + 32, 0:cw],
                                   rf[64 * gi:64 * gi + 32, 0, 0:cw])
                else:
                    nc.vector.tensor_copy(rfsb[64 * gi:64 * gi + 32, 0:cw],
                                          rf[64 * gi:64 * gi + 32, 0, 0:cw])
            oT0, oT1 = oT_tiles[ci]
            dst = oT0 if g < 2 else oT1
            d0 = 64 * (g % 2)
            for gi in range(2):
                nc.vector.tensor_mul(dst[d0 + 32 * gi:d0 + 32 * gi + 32, 0:cw],
                                     av[64 * gi:64 * gi + 32, 0:cw],
                                     rfsb[64 * gi:64 * gi + 32, 0:cw])

        def outproj(ci, s):
            qoff, cw = QCH[ci]
            oT0, oT1 = oT_tiles[ci]
            yps = mscp.tile([128, 2, 512], F32, tag="ms", name="yps")
            nc.tensor.matmul(yps[:, 0, :C], oT0[:, ts_(s, 128)], WF(W_O, 0),
                             start=True, stop=False)
            nc.tensor.matmul(yps[:, 0, :C], oT1[:, ts_(s, 128)], WF(W_O, 1),
                             start=False, stop=True)
            ysb = ypool.tile([128, C], F32, tag="y", name="ysb")
            nc.vector.tensor_add(ysb[:], yps[:, 0, 0:C], bob[:])
            nc.sync.dma_start(io["y"][qoff + 128 * s: qoff + 128 * (s + 1), :],
                              ysb[:])

        delayed = []

        def emit_av(ci, g, t, ptA, ptB):
            qoff, cw = QCH[ci]
            if t == 0:
                av_tiles[(ci, g)] = avp.tile([128, 512], F32, tag="av", name="av")
            av = av_tiles[(ci, g)]
            for gi in range(2):
                h = 2 * g + gi
                vslice = vsb[:, (t * NH + h) * VW:(t * NH + h + 1) * VW]
                if gi == 1 and ptB is not None:
                    pts = (ptB[:, 0:cw].bitcast(F16)
                           if ptB.dtype == I16 else ptB[:, 0:cw])
                else:
                    pts = ptA[:, gi, 0:cw]
                nc.tensor.matmul(
                    av[64 * gi:64 * gi + VW, 0:cw], vslice, pts,
                    start=(t == 0), stop=(t == NT - 1),
                    skip_group_check=True)
            if t == NT - 1:
                fin_a(ci, g)
                delayed.append([1, (fin_b, ci, g)])

        def run_delayed():
            for item in list(delayed):
                item[0] -= 1
                if item[0] <= 0:
                    fn, *args = item[1]
                    fn(*args)
                    delayed.remove(item)

        # ---- filler schedule --------------------------------------------
        fillers = {}

        def addf(ci, g, t, fn, *args):
            fillers.setdefault((ci, g, t), []).append((fn, args))

        for j in range(4):                       # K cg0 chunks 1..4
            addf(0, 0, j, k_unit, 0, j + 1)
        for j in range(4, NT):                   # V tiles 4..17
            addf(0, 0, j - 1, v_unit, j)
        for i in range(5):                       # K cg1 chunks 0..4
            addf(0, 1, 2 * i, k_unit, 1, i)
        addf(0, 1, 10, q_unit, 1, 0)             # Q cg1 chunk0
        addf(0, 2, 0, q_unit, 0, 1)
        addf(0, 2, 2, q_unit, 1, 1)
        addf(1, 2, 0, q_unit, 0, 2)
        addf(1, 2, 2, q_unit, 1, 2)
        for s in range(4):                       # out-proj of previous chunk
            addf(1, 0, 2 + 2 * s, outproj, 0, s)
            addf(2, 0, 2 + 2 * s, outproj, 1, s)

        # ---- pre-attention head: minimal work to start scores ------------
        k_unit(0, 0)
        q_unit(0, 0)
        for t in range(4):
            v_unit(t)

        # ---- pipelined attention (A@V runs two slots behind scores) ------
        pending = deque()
        for ci in range(3):
            qoff, cw = QCH[ci]
            for g in range(NG):
                for t in range(NT):
                    if g == 0 and t == 0:
                        oT_tiles[ci] = (
                            outpool.tile([128, 512], F16, tag="o0", name="oT0"),
                            outpool.tile([128, 512], F16, tag="o1", name="oT1"))
                    # Scores for both heads into one [128, 2, 512] tile
                    # (each gi slice is one aligned PSUM bank), then ONE
                    # 1024-element exp on ScalarE: the ~475ns real
                    # per-instruction ScalarE overhead makes instruction
                    # count the dominant cost, so never split the exp.
                    ptB = None
                    sc = scpA.tile([128, 2, 512], F32, tag="sc0", name="sc")
                    ptA = ptpool.tile([128, 2, 512], F16, tag="ptA",
                                      name="ptA")
                    for gi in range(2):
                        h = 2 * g + gi
                        cg, hh = h // 4, h % 4
                        nc.tensor.matmul(
                            sc[:, gi, 0:cw],
                            kT[cg][ts_(hh, 32), ts_(t, 128)],
                            qT[cg][ts_(hh, 32), qoff:qoff + cw],
                            start=True, stop=True, tile_position=(32 * hh, 0))
                    nc.scalar.activation(ptA[:, 0:2, 0:cw],
                                         sc[:, 0:2, 0:cw], AF.Exp)
                    run_delayed()
                    depth = 2 if pipeline else 0
                    while len(pending) >= max(depth, 1):
                        emit_av(*pending.popleft())
                        if not pipeline:
                            run_delayed()
                    pending.append((ci, g, t, ptA, ptB))
                    if not pipeline:
                        emit_av(*pending.popleft())
                        run_delayed()
                    for fn, args in fillers.get((ci, g, t), []):
                        fn(*args)
        while pending:
            emit_av(*pending.popleft())
            run_delayed()
        run_delayed()          # fin_b of (2, 3)
        outproj(2, 0)

        if R > 1:
            loop_ctx.__exit__(None, None, None)


def build_nc(R=1, act_copy=False, schraudolph=False, act_dma=True,
             pipeline=True, split=False):
    nc = bacc.Bacc("TRN2", target_bir_lowering=False, debug=False,
                   enable_asserts=True, num_devices=8)
    io = {}
    for name, shape, dt in IN_SPECS:
        io[name] = nc.dram_tensor(name, shape, dt, kind="ExternalInput").ap()
    io["y"] = nc.dram_tensor("y", [NQ, C], F32, kind="ExternalOutput").ap()

    with tile.TileContext(nc) as tc:
        emit(tc, io, R=R, act_copy=act_copy, schraudolph=schraudolph,
             act_dma=act_dma, pipeline=pipeline, split=split)
    nc.compile()
    return nc


def host_inputs(x, Wq, q_bias, Wk, Wv, v_bias, Wo, bo):
    """Build the per-core input maps (host-side sharding + layout prep)."""
    xf = np.ascontiguousarray(x.reshape(B, N, C))

    inv_freq = 1.0 / (ROPE_BASE ** (np.arange(0, HD, 2, dtype=np.float64) / HD))
    pos = np.arange(N, dtype=np.float64)
    ang = pos[:, None] * inv_freq[None, :]          # [N, 16]
    cos_t, sin_t = np.cos(ang), np.sin(ang)         # [N, 16]
    # channel c -> within-head index jj = c % 32, freq f = jj % 16
    jj = np.arange(C) % HD
    f = jj % D2
    CT = cos_t[:, f].T                              # [C, N] float64
    ST = sin_t[:, f].T

    # signed rotate-half permutation RM [C, C]: partner = RM @ q
    RM = np.zeros((C, C), dtype=np.float64)
    for p in range(C):
        j = p % HD
        if j < D2:
            RM[p, p + D2] = -1.0                    # partner[p] = -q[p+16]
        else:
            RM[p, p - D2] = 1.0                     # partner[p] = +q[p-16]

    Wq64, Wk64 = Wq.astype(np.float64), Wk.astype(np.float64)
    Wqr = RM @ Wq64                                 # rotated projections
    Wkr = RM @ Wk64
    rqb = RM @ q_bias.astype(np.float64)

    f16 = lambda a: np.ascontiguousarray(a, dtype=np.float16)
    f32 = lambda a: np.ascontiguousarray(a, dtype=np.float32)

    halves = []
    for M in (Wk64.T, Wkr.T, Wq64.T, Wqr.T, Wv.T, Wo.T):
        halves.append(np.asarray(M[0:128, :], np.float64))
        halves.append(np.asarray(M[128:256, :], np.float64))
    wpack = f16(np.concatenate(halves, axis=1))     # [128, 12*256]
    bpack = f32(np.stack([q_bias[0:128], q_bias[128:256],
                          rqb[0:128], rqb[128:256]], axis=1))
    # v_bias folded through the output projection: attn weights sum to 1
    # after normalization, so out = attn@(V+vb) = attn@V + vb exactly.
    bob = Wo.astype(np.float64) @ v_bias.astype(np.float64) + bo

    common = {
        "wpack": wpack, "bpack": bpack,
        "ones": np.ones((128, 128), dtype=np.float16),
        "bob": f32(np.broadcast_to(bob, (128, C))),
        "CTK": f16(CT), "STK": f16(ST),
    }
    in_maps = []
    for core in range(8):
        b, qhalf = core // 2, core % 2
        qoff = qhalf * NQ
        xT = xf[b].T
        m = dict(common)
        m["xT"] = f16(xT)
        m["xTq"] = f16(xT[:, qoff:qoff + NQ])
        m["CTQ"] = f16(CT[:, qoff:qoff + NQ] * SCALE)
        m["STQ"] = f16(ST[:, qoff:qoff + NQ] * SCALE)
        in_maps.append(m)
    return in_maps


_NC_CACHE = {}


def get_nc(R=1):
    if R not in _NC_CACHE:
        _NC_CACHE[R] = build_nc(R)
    return _NC_CACHE[R]


def kernel(**inputs):
    inputs = {k: np.asarray(v, dtype=np.float32) for k, v in inputs.items()}
    in_maps = host_inputs(**inputs)
    nc = get_nc()
    res = run_bass_kernel_spmd(nc, in_maps, core_ids=list(range(8)))
    out = np.empty((B, N, C), dtype=np.float32)
    for core in range(8):
        b, qhalf = core // 2, core % 2
        qoff = qhalf * NQ
        out[b, qoff:qoff + NQ, :] = res.results[core]["y"]
    return out.reshape(B, HH, WW, C)
